# revision 5
# baseline (speedup 1.0000x reference)
"""Trainium2 Bass kernel for JinaEmbeddingsV3 self-attention with per-batch LoRA.

Sharding: data-parallel over batch (B=8 -> 8 cores, one batch row each).

Per-core dataflow (everything in "T layout", i.e. feature dim on partitions):
  - host feeds hsT = hidden[b].T, wT = W.T (q/k output rows permuted so each
    head's RoPE halves land in separate 128-row tile groups), lora A/B gathered
    per batch and transposed, cos/sin tables tiled to [128, S].
  - loT[r,s]   = A_b.T.T @ hsT          (rank-4, K=128 chunks)
  - qkT[o,s]   = sum_d wT[d,o].T @ hsT[d,s] + lora + bias   (o on partitions)
  - v[s,o]     = sum_d hsT[d,s].T @ wT[d,ov] + lora + bias  (natural layout,
                 stored with a ones-column gap after each head's 64 cols)
  - RoPE on qkT in-place (half-split permutation makes rotate_half a plain
    full-tile multiply against the partner tile group)
  - per head: scoresT[kk,q] = k_h @ q_h.T via two K=32 row-packed matmuls,
    exp on ACT (scale=1/8, bias=attention-mask column) -> expT
    avT: av[q,hd]+denominator via lhsT=[v_h|ones] (M=65) accumulated over kk
    normalize with DVE reciprocal + GPSIMD partition broadcast
  - output written transposed (outT[o,s]); host transposes back.

All matmuls run as float32r (1 cycle/row at N>=256 vs 4 for plain fp32).
"""

import os
import sys

import numpy as np

for _p in ("/opt/trn_rl_repo",):
    if _p not in sys.path and os.path.isdir(_p):
        sys.path.insert(0, _p)

S = 1024
D = 1024
H = 16
HD = 64
O3 = 3 * D
R = 4
P = 128
B = 8
NQ = 2  # 512-wide free-dim chunks per 1024
DT = D // P  # 8 contraction chunks
LORA_SCALING = 0.25
VROW = H * (HD + 1)  # v_sb row width: 64 data cols + 1 ones col per head


def _qk_perm() -> np.ndarray:
    """New row r -> original in-head component index (o = h*64 + j).

    First 512 rows hold every head's first RoPE half (j<32), second 512 rows
    the second half, so rotate_half pairs sit at identical partition offsets
    in tile groups 0-3 vs 4-7.
    """
    perm = np.zeros(1024, np.int64)
    for r in range(512):
        h, j = r // 32, r % 32
        perm[r] = h * 64 + j
    for r in range(512, 1024):
        h, j = (r - 512) // 32, 32 + (r - 512) % 32
        perm[r] = h * 64 + j
    return perm


_PERM = _qk_perm()


def _build_program():
    import concourse.bass as bass  # noqa: F401
    import concourse.tile as tile
    from concourse import bacc, mybir
    from contextlib import ExitStack

    f32 = mybir.dt.float32
    f32r = mybir.dt.float32r
    Alu = mybir.AluOpType
    Act = mybir.ActivationFunctionType

    nc = bacc.Bacc(None, target_bir_lowering=False, debug=False)

    hsT = nc.declare_dram_parameter("hsT", [D, S], f32r, isOutput=False)
    wT = nc.declare_dram_parameter("wT", [D, O3], f32r, isOutput=False)
    biasqk = nc.declare_dram_parameter("biasqk", [P, 16], f32, isOutput=False)
    biasv = nc.declare_dram_parameter("biasv", [P, D], f32, isOutput=False)
    maskc = nc.declare_dram_parameter("maskc", [P, DT], f32, isOutput=False)
    cosb = nc.declare_dram_parameter("cosb", [P, S], f32, isOutput=False)
    sinb = nc.declare_dram_parameter("sinb", [P, S], f32, isOutput=False)
    abT = nc.declare_dram_parameter("abT", [D, R], f32r, isOutput=False)
    bbT = nc.declare_dram_parameter("bbT", [R, O3], f32r, isOutput=False)
    outT = nc.declare_dram_parameter("outT", [D, S], f32, isOutput=True)

    def mm(out, lhsT, rhs, **kw):
        nc.tensor.matmul(out, lhsT, rhs, **kw)

    with tile.TileContext(nc) as tc, ExitStack() as stack:
        persist = stack.enter_context(tc.tile_pool(name="persist", bufs=1))

        cos_sb = persist.tile([P, S], f32, tag="cos")
        sin_sb = persist.tile([P, S], f32, tag="sin")
        biasqk_sb = persist.tile([P, 16], f32, tag="biasqk")
        biasv_sb = persist.tile([P, D], f32, tag="biasv")
        maskc_sb = persist.tile([P, DT], f32, tag="maskc")
        abT_sb = persist.tile([P, DT, R], f32r, tag="abT")
        bbT_sb = persist.tile([R, O3], f32r, tag="bbT")
        loT_sb = persist.tile([R, S], f32r, tag="loT")
        qk_sb = [persist.tile([P, S], f32r, tag=f"qk{t}", name=f"qk{t}") for t in range(16)]
        v_sb = [persist.tile([P, VROW], f32r, tag=f"v{t}", name=f"v{t}") for t in range(DT)]

        nc.sync.dma_start(cos_sb[:], cosb[:])
        nc.sync.dma_start(sin_sb[:], sinb[:])
        nc.sync.dma_start(biasqk_sb[:], biasqk[:])
        nc.sync.dma_start(biasv_sb[:], biasv[:])
        nc.sync.dma_start(maskc_sb[:], maskc[:])
        nc.sync.dma_start(abT_sb[:], abT.rearrange("(c p) r -> p c r", p=P))
        nc.sync.dma_start(bbT_sb[:], bbT[:])

        # ones columns in the gapped v layout (one per head, col 64 of 65).
        # memset can't write f32r; DVE copy from an f32 ones column rounds.
        ones_col = persist.tile([P, 1], f32, tag="ones")
        nc.vector.memset(ones_col[:], 1.0)
        for t in range(DT):
            nc.vector.tensor_copy(
                v_sb[t][:].rearrange("p (h c) -> p h c", c=HD + 1)[:, :, HD : HD + 1],
                ones_col[:, 0:1].rearrange("p (a b) -> p a b", b=1).to_broadcast(
                    (P, H, 1)
                ),
            )

        with (
            tc.tile_pool(name="proj_sbuf", bufs=1) as proj_sbuf,
            tc.tile_pool(name="wqk_pool", bufs=16) as wqk_pool,
            tc.tile_pool(name="wv_pool", bufs=8) as wv_pool,
            tc.tile_pool(name="proj_psum", bufs=4, space="PSUM") as proj_psum,
            tc.tile_pool(name="lo_psum", bufs=2, space="PSUM") as lo_psum,
            tc.tile_pool(name="rope_tmp", bufs=2) as rope_tmp,
        ):
            hs_sb = [proj_sbuf.tile([P, S], f32r, tag=f"hs{t}", name=f"hs{t}") for t in range(DT)]
            for t in range(DT):
                nc.sync.dma_start(hs_sb[t][:], hsT[t * P : (t + 1) * P, :])

            # ---- loT = (A_b @ hs.T) : [R, S], scaled by LORA_SCALING ----
            for n in range(NQ):
                ps = lo_psum.tile([R, 512], f32, tag="lo")
                for c in range(DT):
                    mm(
                        ps[:],
                        abT_sb[:, c, :],
                        hs_sb[c][:, n * 512 : (n + 1) * 512],
                        start=(c == 0),
                        stop=(c == DT - 1),
                    )
                nc.scalar.mul(
                    loT_sb[:, n * 512 : (n + 1) * 512], ps[:], LORA_SCALING
                )

            # ---- qkT projection: 16 o-chunks of 128 rows ----
            for o in range(16):
                wts = []
                for dch in range(DT):
                    wt = wqk_pool.tile([P, P], f32r, tag="wqk", name=f"wqk_{o}_{dch}")
                    nc.sync.dma_start(
                        wt[:],
                        wT[dch * P : (dch + 1) * P, o * P : (o + 1) * P],
                    )
                    wts.append(wt)
                for n in range(NQ):
                    ps = proj_psum.tile([P, 512], f32, tag="proj")
                    for dch in range(DT):
                        mm(
                            ps[:],
                            wts[dch][:],
                            hs_sb[dch][:, n * 512 : (n + 1) * 512],
                            start=(dch == 0),
                            stop=False,
                        )
                    mm(
                        ps[:],
                        bbT_sb[:, o * P : (o + 1) * P],
                        loT_sb[:, n * 512 : (n + 1) * 512],
                        start=False,
                        stop=True,
                    )
                    nc.vector.tensor_scalar_add(
                        qk_sb[o][:, n * 512 : (n + 1) * 512],
                        ps[:],
                        biasqk_sb[:, o : o + 1],
                    )

            # ---- RoPE in place on qkT ----
            # F' = F*cos - S*sin ; S' = S*cos + F*sin
            for base in (0, 8):  # q tiles 0-7, k tiles 8-15
                for t in range(4):
                    Ft = qk_sb[base + t]
                    St = qk_sb[base + 4 + t]
                    t1 = rope_tmp.tile([P, S], f32, tag="ropetmp")
                    t2 = rope_tmp.tile([P, S], f32, tag="ropetmp")
                    nc.vector.tensor_mul(out=t1[:], in0=St[:], in1=sin_sb[:])
                    nc.vector.tensor_mul(out=t2[:], in0=Ft[:], in1=sin_sb[:])
                    nc.vector.tensor_mul(out=Ft[:], in0=Ft[:], in1=cos_sb[:])
                    nc.vector.tensor_tensor(Ft[:], Ft[:], t1[:], Alu.subtract)
                    nc.vector.tensor_mul(out=St[:], in0=St[:], in1=cos_sb[:])
                    nc.vector.tensor_tensor(St[:], St[:], t2[:], Alu.add)

            # ---- v projection (natural [s, ov] layout, gapped columns) ----
            for n in range(NQ):
                wvs = []
                for dch in range(DT):
                    wv = wv_pool.tile([P, 512], f32r, tag="wv", name=f"wv_{n}_{dch}")
                    nc.sync.dma_start(
                        wv[:],
                        wT[
                            dch * P : (dch + 1) * P,
                            2 * D + n * 512 : 2 * D + (n + 1) * 512,
                        ],
                    )
                    wvs.append(wv)
                for sch in range(DT):
                    ps = proj_psum.tile([P, 512], f32, tag="proj")
                    for dch in range(DT):
                        mm(
                            ps[:],
                            hs_sb[dch][:, sch * P : (sch + 1) * P],
                            wvs[dch][:],
                            start=(dch == 0),
                            stop=False,
                        )
                    mm(
                        ps[:],
                        loT_sb[:, sch * P : (sch + 1) * P],
                        bbT_sb[:, 2 * D + n * 512 : 2 * D + (n + 1) * 512],
                        start=False,
                        stop=True,
                    )
                    dst = v_sb[sch][:].rearrange("p (h c) -> p h c", c=HD + 1)[
                        :, n * 8 : (n + 1) * 8, 0:HD
                    ]
                    nc.vector.tensor_tensor(
                        dst,
                        ps[:].rearrange("p (h c) -> p h c", c=HD),
                        biasv_sb[:, n * 512 : (n + 1) * 512].rearrange(
                            "p (h c) -> p h c", c=HD
                        ),
                        Alu.add,
                    )

        # ---- attention ----
        with (
            tc.tile_pool(name="exp_pool", bufs=8) as exp_pool,
            tc.tile_pool(name="recip_pool", bufs=4) as recip_pool,
            tc.tile_pool(name="rbc_pool", bufs=4) as rbc_pool,
            tc.tile_pool(name="out_pool", bufs=6) as out_pool,
            tc.tile_pool(name="sc_psum", bufs=4, space="PSUM") as sc_psum,
            tc.tile_pool(name="av_psum", bufs=4, space="PSUM") as av_psum,
        ):
            for g in range(4):  # head groups of 4 (rows 32*hh in the F/S tiles)
                outs = [out_pool.tile([HD, S], f32, tag="out", name=f"out_g{g}_{i}") for i in range(4)]
                for n in range(NQ):
                    qs = slice(n * 512, (n + 1) * 512)
                    av_ps = [av_psum.tile([65, 512], f32, tag="av", name=f"av_{g}_{n}_{i}") for i in range(4)]
                    for c in range(DT):
                        ks = slice(c * P, (c + 1) * P)
                        sc_ps = [
                            sc_psum.tile([P, 512], f32, tag="sc", name=f"sc_{g}_{n}_{c}_{i}")
                            for i in range(4)
                        ]
                        for hh in range(4):
                            rp = 32 * hh
                            mm(
                                sc_ps[hh][:],
                                qk_sb[8 + g][rp : rp + 32, ks],
                                qk_sb[g][rp : rp + 32, qs],
                                start=True,
                                stop=False,
                                tile_position=(rp, 0),
                            )
                        for hh in range(4):
                            rp = 32 * hh
                            mm(
                                sc_ps[hh][:],
                                qk_sb[12 + g][rp : rp + 32, ks],
                                qk_sb[4 + g][rp : rp + 32, qs],
                                start=False,
                                stop=True,
                                tile_position=(rp, 0),
                            )
                        for hh in range(4):
                            h = 4 * g + hh
                            et = exp_pool.tile([P, 512], f32r, tag="exp")
                            nc.scalar.activation(
                                et[:],
                                sc_ps[hh][:],
                                Act.Exp,
                                bias=maskc_sb[:, c : c + 1],
                                scale=float(HD) ** -0.5,
                            )
                            mm(
                                av_ps[hh][:],
                                v_sb[c][:, h * (HD + 1) : (h + 1) * (HD + 1)],
                                et[:],
                                start=(c == 0),
                                stop=(c == DT - 1),
                            )
                    for hh in range(4):
                        rc = recip_pool.tile([1, 512], f32, tag="recip")
                        nc.vector.reciprocal(rc[:], av_ps[hh][64:65, :])
                        rb = rbc_pool.tile([HD, 512], f32, tag="rbc")
                        nc.gpsimd.partition_broadcast(rb[:], rc[:])
                        nc.vector.tensor_mul(
                            out=outs[hh][:, qs],
                            in0=av_ps[hh][0:HD, :],
                            in1=rb[:],
                        )
                for hh in range(4):
                    h = 4 * g + hh
                    nc.sync.dma_start(outT[h * HD : (h + 1) * HD, :], outs[hh][:])

    nc.compile()
    return nc


_CACHED_NC = None


def _get_program():
    global _CACHED_NC
    if _CACHED_NC is None:
        _CACHED_NC = _build_program()
    return _CACHED_NC


def _prep_maps(inputs: dict) -> list[dict]:
    hs = np.asarray(inputs["hidden_states"], np.float32)
    mask = np.asarray(inputs["attention_mask"], np.float32)
    cos = np.asarray(inputs["cos"], np.float32)
    sin = np.asarray(inputs["sin"], np.float32)
    am = np.asarray(inputs["adapter_mask"]).astype(np.int64)
    W = np.asarray(inputs["Wqkv_weight"], np.float32)
    bias = np.asarray(inputs["Wqkv_bias"], np.float32)
    lA = np.asarray(inputs["lora_A"], np.float32)
    lB = np.asarray(inputs["lora_B"], np.float32)

    Wnew = np.concatenate([W[:D][_PERM], W[D : 2 * D][_PERM], W[2 * D :]], 0)
    wT = np.ascontiguousarray(Wnew.T)
    bqk = np.concatenate([bias[:D][_PERM], bias[D : 2 * D][_PERM]])
    biasqk = np.ascontiguousarray(bqk.reshape(16, P).T)
    biasv = np.ascontiguousarray(
        np.broadcast_to(bias[2 * D :][None, :], (P, D))
    )

    maps = []
    for b in range(B):
        hsT = np.ascontiguousarray(hs[b].T)
        cosb = np.ascontiguousarray(np.tile(cos[b].T[:32], (4, 1)))
        sinb = np.ascontiguousarray(np.tile(sin[b].T[:32], (4, 1)))
        maskc = np.ascontiguousarray(mask[b, 0, 0].reshape(DT, P).T)
        a = lA[am[b]]
        Bm = lB[am[b]]
        Bnew = np.concatenate(
            [Bm[:D][_PERM], Bm[D : 2 * D][_PERM], Bm[2 * D :]], 0
        )
        maps.append(
            dict(
                hsT=hsT,
                wT=wT,
                biasqk=biasqk,
                biasv=biasv,
                maskc=maskc,
                cosb=cosb,
                sinb=sinb,
                abT=np.ascontiguousarray(a.T),
                bbT=np.ascontiguousarray(Bnew.T),
            )
        )
    return maps


def kernel(**inputs) -> np.ndarray:
    from concourse.bass_utils import run_bass_kernel_spmd

    nc = _get_program()
    in_maps = _prep_maps(inputs)
    res = run_bass_kernel_spmd(nc, in_maps, core_ids=list(range(B)))
    out = np.stack(
        [np.asarray(res.results[b]["outT"], np.float32).T for b in range(B)]
    )
    return np.ascontiguousarray(out)
